# revision 8
# baseline (speedup 1.0000x reference)
"""Multi-head causal attention (B=4, T=2048, C=1024, H=16) on 8 TRN2 NeuronCores.

Sharding: core c handles batch b = c//2 and head-group g = c%2 (8 heads,
512 output channels). Host passes per-core transposed operands (x[b].T and
W[rows].T) so every on-device matmul streams natural-layout tiles:

  q.T[d,i] = sum_c WqT[c,d] * xT[c,i]   -> matmul(lhsT=WqT tile, rhs=xT tile)
  v[t,e]   = sum_c xT[c,t]  * WvT[c,e]  -> matmul(lhsT=xT tile,  rhs=WvT tile)
  S.T[j,i] = sum_d kT[d,j]  * qT[d,i]   -> matmul(lhsT=kT tile,  rhs=qT tile)
  O.T[e,i] = sum_j v[j,e]   * P[j,i]    -> matmul(lhsT=v tile,   rhs=exp tile)

V carries an appended ones column per head so row 64 of the O.T accumulator
is the softmax denominator; causal masking is a 0/1 multiply on the exp tile
(exp(-inf) == 0), and the reference's mask-then-scale equals scale-then-mask
here because masked lanes are zeroed after exp. No max-subtraction pass:
scores*scale ~ N(0,1), so exp stays comfortably inside f32/bf16 range.
"""
import numpy as np

import bass_rust
import concourse.bass as bass
import concourse.mybir as mybir
import concourse.tile as tile
from concourse.bass_utils import run_bass_kernel_spmd

P = 128
HS = 64  # head size


def _split_sync_waits(nc, max_waits=1):
    # This walrus build's setupSyncWait admits a single sync-wait slot per
    # instruction, but Tile can emit several (cross-proc deps on one inst).
    # Peel extra waits onto preceding same-engine NOPs (pure wait carriers;
    # a Drain would flush the PE pipe).
    all_bbs = [b for fn in nc.m.functions for b in fn.blocks]
    for bb in all_bbs:
        insts = bb.instructions
        i = 0
        while i < len(insts):
            inst = insts[i]
            si = inst.sync_info
            ow = list(si.on_wait) if si and si.on_wait else []
            if len(ow) > max_waits:
                keep = ow[-max_waits:]
                rest = ow[:-max_waits]
                eng = nc.engines[inst.engine]
                new_insts = []
                while rest:
                    chunk, rest = rest[:max_waits], rest[max_waits:]
                    d = eng.nop()
                    d.ins.sync_info = bass_rust.SyncInfo(on_wait=chunk, on_update=[])
                    new_insts.append(d.ins)
                for bb2 in all_bbs:
                    ilist = bb2.instructions
                    changed = False
                    for ni in new_insts:
                        if ni in ilist:
                            ilist.remove(ni)
                            changed = True
                    if changed:
                        bb2.instructions = ilist
                si.on_wait = keep
                bb.instructions = insts[:i] + new_insts + insts[i:]
                insts = bb.instructions
                i += len(new_insts)
            i += 1
    return nc


def _make_masks(fb):
    # mask[l, p, i] = 1 if key (128*l + p) <= query i else 0, bf16
    import ml_dtypes

    lt = fb // P
    j = (np.arange(lt * P)[:, None]).reshape(lt, P, 1)
    i = np.arange(fb)[None, None, :]
    return (j <= i).astype(ml_dtypes.bfloat16)


def build_nc(T=2048, C=1024, D=512, FB=512):
    """One-core SPMD program: xT (C,T), wqT/wkT/wvT (C,D) -> out_t (D,T)."""
    f32 = mybir.dt.float32
    bf16 = mybir.dt.bfloat16
    CK = C // P  # contraction subtiles
    DT = D // P  # q/k d-tiles
    TT = T // P  # t-tiles (v rows / key tiles)
    TB = T // FB  # query blocks
    JB = FB // P  # key tiles per query block
    H = D // HS  # local heads
    HPD = P // HS  # heads per d-tile (2)
    scale = float(HS) ** -0.5

    nc = bass.Bass()
    xT = nc.declare_dram_parameter("xT", [C, T], f32, isOutput=False)
    wqT = nc.declare_dram_parameter("wqT", [C, D], f32, isOutput=False)
    wkT = nc.declare_dram_parameter("wkT", [C, D], f32, isOutput=False)
    wvT = nc.declare_dram_parameter("wvT", [C, D], f32, isOutput=False)
    out_t = nc.declare_dram_parameter("out_t", [D, T], f32, isOutput=True)
    masks = nc.inline_tensor(_make_masks(FB), name="causal_masks")
    ones = nc.inline_tensor(np.ones((1, HS), np.float32), name="ones_row")

    xT_r = xT.rearrange("(ck p) t -> p ck t", p=P)
    wT_r = {w.name: w.rearrange("(ck p) d -> p ck d", p=P) for w in (wqT, wkT, wvT)}

    with tile.TileContext(nc) as tc:
        with (
            tc.tile_pool(name="persist", bufs=1) as persist,
            tc.tile_pool(name="stage", bufs=3) as stage,
            tc.tile_pool(name="work", bufs=3) as work,
            tc.tile_pool(name="ps_proj", bufs=2, space="PSUM") as ps_proj,
            tc.tile_pool(name="ps_s", bufs=2, space="PSUM") as ps_s_pool,
            tc.tile_pool(name="ps_o", bufs=2, space="PSUM") as ps_o_pool,
            tc.tile_pool(name="ps_rec", bufs=2, space="PSUM") as ps_rec_pool,
        ):
            # ---- load + cast inputs to bf16 ----
            x_bf = persist.tile([P, CK, T], bf16, tag="x_bf")
            for ck in range(CK):
                st = stage.tile([P, T], f32, tag="x_stage")
                nc.sync.dma_start(st[:], xT_r[:, ck, :])
                nc.vector.tensor_copy(x_bf[:, ck, :], st[:])

            w_bf = {}
            for w in (wqT, wkT, wvT):
                dst = persist.tile([P, CK, D], bf16, tag=f"{w.name}_bf")
                w_bf[w.name] = dst
                for ck in range(CK):
                    st = stage.tile([P, D], f32, tag="w_stage")
                    nc.sync.dma_start(st[:], wT_r[w.name][:, ck, :])
                    nc.vector.tensor_copy(dst[:, ck, :], st[:])

            mask_sb = persist.tile([P, JB, FB], bf16, tag="mask_sb")
            nc.sync.dma_start(mask_sb[:], masks[:].rearrange("l p i -> p l i"))
            ones_sb = persist.tile([1, HS], f32, tag="ones_sb")
            nc.sync.dma_start(ones_sb[:], ones[:])

            # ---- projections ----
            qT = persist.tile([P, DT, T], bf16, tag="qT")
            kT = persist.tile([P, DT, T], bf16, tag="kT")
            for w, dst in ((wqT, qT), (wkT, kT)):
                for dt in range(DT):
                    for bi in range(TB):
                        ps = ps_proj.tile([P, FB], f32, tag="ps_proj")
                        for ck in range(CK):
                            nc.tensor.matmul(
                                ps[:],
                                lhsT=w_bf[w.name][:, ck, dt * P : (dt + 1) * P],
                                rhs=x_bf[:, ck, bi * FB : (bi + 1) * FB],
                                start=(ck == 0),
                                stop=(ck == CK - 1),
                            )
                        nc.vector.tensor_copy(dst[:, dt, bi * FB : (bi + 1) * FB], ps[:])

            # v with an appended ones column per head: [P, TT, H*(HS+1)]
            v_sb = persist.tile([P, TT, H * (HS + 1)], bf16, tag="v_sb")
            for tt in range(TT):
                ps = ps_proj.tile([P, D], f32, tag="ps_proj")
                for ck in range(CK):
                    nc.tensor.matmul(
                        ps[:],
                        lhsT=x_bf[:, ck, tt * P : (tt + 1) * P],
                        rhs=w_bf[wvT.name][:, ck, :],
                        start=(ck == 0),
                        stop=(ck == CK - 1),
                    )
                v3 = v_sb[:, tt, :].rearrange("p (h e) -> p h e", e=HS + 1)
                nc.vector.tensor_copy(v3[:, :, 0:HS], ps[:].rearrange("p (h e) -> p h e", e=HS))
                nc.vector.memset(v3[:, :, HS : HS + 1], 1.0)

            # ---- causal attention, head-by-head ----
            for h in range(H):
                dt = h // HPD
                po = HS * (h % HPD)
                for bi in range(TB):
                    nj = (bi + 1) * JB
                    ps_o = ps_o_pool.tile([HS + 1, FB], f32, tag="ps_o")
                    exps = [None] * nj
                    # software pipeline: S(j)/exp(j) run one step ahead of O(j)
                    for jt in range(nj + 1):
                        if jt < nj:
                            ps_s = ps_s_pool.tile([P, FB], f32, tag="ps_s")
                            nc.tensor.matmul(
                                ps_s[:],
                                lhsT=kT[po : po + HS, dt, jt * P : (jt + 1) * P],
                                rhs=qT[po : po + HS, dt, bi * FB : (bi + 1) * FB],
                                start=True,
                                stop=True,
                            )
                            ex = work.tile([P, FB], bf16, tag="exp")
                            nc.scalar.activation(
                                ex[:], ps_s[:], mybir.ActivationFunctionType.Exp, scale=scale
                            )
                            if jt >= bi * JB:
                                nc.vector.tensor_mul(ex[:], ex[:], mask_sb[:, jt - bi * JB, :])
                            exps[jt] = ex
                        if jt > 0:
                            nc.tensor.matmul(
                                ps_o[:],
                                lhsT=v_sb[:, jt - 1, (HS + 1) * h : (HS + 1) * (h + 1)],
                                rhs=exps[jt - 1][:],
                                start=(jt == 1),
                                stop=(jt == nj),
                            )
                    rec = work.tile([1, FB], f32, tag="rec")
                    nc.vector.reciprocal(rec[:], ps_o[HS : HS + 1, :])
                    # broadcast 1/den across the 64 output partitions via a
                    # K=1 matmul with a ones column (DVE can't stride-0 read)
                    ps_rec = ps_rec_pool.tile([HS, FB], f32, tag="ps_rec")
                    nc.tensor.matmul(ps_rec[:], lhsT=ones_sb[:], rhs=rec[:], start=True, stop=True)
                    rec_sb = work.tile([HS, FB], f32, tag="rec_sb")
                    nc.vector.tensor_copy(rec_sb[:], ps_rec[:])
                    ob = work.tile([HS, FB], f32, tag="ob")
                    nc.vector.tensor_mul(ob[:], ps_o[0:HS, :], rec_sb[:])
                    nc.sync.dma_start(out_t[HS * h : HS * (h + 1), bi * FB : (bi + 1) * FB], ob[:])

    _split_sync_waits(nc)
    return nc


_NC_CACHE = {}


def _get_nc(key=(2048, 1024, 512, 512)):
    if key not in _NC_CACHE:
        _NC_CACHE[key] = build_nc(*key)
    return _NC_CACHE[key]


def run(x, Wq, Wk, Wv, trace=False, **spmd_kwargs):
    B, T, C = x.shape
    n_cores = 8
    gpb = 2  # head-groups per batch
    D = C // gpb

    nc = _get_nc((T, C, D, 512))

    in_maps = []
    for c in range(n_cores):
        b, g = c // gpb, c % gpb
        rows = slice(g * D, (g + 1) * D)
        in_maps.append(
            {
                "xT": np.ascontiguousarray(np.asarray(x)[b].T),
                "wqT": np.ascontiguousarray(np.asarray(Wq)[rows].T),
                "wkT": np.ascontiguousarray(np.asarray(Wk)[rows].T),
                "wvT": np.ascontiguousarray(np.asarray(Wv)[rows].T),
            }
        )

    res = run_bass_kernel_spmd(
        nc, in_maps, core_ids=list(range(n_cores)), trace=trace, **spmd_kwargs
    )

    out = np.empty((B, T, C), np.float32)
    for c in range(n_cores):
        b, g = c // gpb, c % gpb
        out[b, :, g * D : (g + 1) * D] = res.results[c]["out_t"].T
    return out, res


def kernel(x, Wq, Wk, Wv, **_):
    out, _res = run(x, Wq, Wk, Wv, trace=False)
    return out
